# revision 1
# baseline (speedup 1.0000x reference)
"""Trainium2 Bass kernel for nn_Decoder_74835510165950 (sparse_attention).

Single-query attention decoder over B=64, N=2000, H=128, 8 heads.
Data-parallel over 8 NeuronCores: 8 batches per core.

Algebraic restructuring (q_len = 1 makes K/V materialization useless):
  scores[b,h,n] = X_b[n,:] @ R_b[:,h]      with R_b = Wk^T @ (blockdiag q~_b)
  attn_out u    = X_b^T @ attn_b           then per-head Wv fold
  pointer[b,n]  = X_b[n,:] @ w_b           with w_b = logit_Wk^T @ fq_b / sqrt(H)
so per core we read X once (8 MB) and run three PE streams over it.

Layout per core (b = 0..7 -> quad q = b // 4, bi = b % 4):
  score/pointer PSUM tiles [128, 2000]: batch bi occupies rows 32*bi..32*bi+8
  (engine APs require 32-aligned partition bases). -1e9 masks are folded into
  the PSUM accumulation via one extra matmul with a host one-hot rhs.
fp32r (TF32-class, 1 cyc/row) for big streams; fp32 for small matmuls.
"""
import sys

if "/opt/trn_rl_repo" not in sys.path:
    sys.path.insert(0, "/opt/trn_rl_repo")

import math
import numpy as np

import concourse.bass as bass
import concourse.tile as tile
from concourse import bacc, mybir
from concourse.bass_utils import run_bass_kernel_spmd

F32 = mybir.dt.float32
F32R = mybir.dt.float32r
BF16 = mybir.dt.bfloat16
I32 = mybir.dt.int32

N_CORES = 8
B_CORE = 8          # batches per core
N = 2000
H = 128
NH = 8              # heads
HD = 16             # head dim
SCHUNKS = [(0, 512), (512, 512), (1024, 512), (1536, 464)]  # bank-aligned
NCHUNK = 500        # xT copy chunk
NJ = 16             # n-chunks per batch
NP = 125            # rows per n-chunk (16 * 125 = 2000)

_CACHE = {}


def r(ap):
    return ap.bitcast(F32R)


def build():
    nc = bacc.Bacc("TRN2", target_bir_lowering=False, debug=False)

    x = nc.dram_tensor("x", [B_CORE, N, H], F32, kind="ExternalInput")
    clsT = nc.dram_tensor("clsT", [H, B_CORE], F32, kind="ExternalInput")
    wqgT = nc.dram_tensor("wqgT", [H, H], F32, kind="ExternalInput")
    wsumT = nc.dram_tensor("wsumT", [H, H], F32, kind="ExternalInput")
    wk = nc.dram_tensor("wk", [H, H], F32, kind="ExternalInput")
    wvT = nc.dram_tensor("wvT", [H, H], F32, kind="ExternalInput")
    wcT = nc.dram_tensor("wcT", [H, H], F32, kind="ExternalInput")
    wlk = nc.dram_tensor("wlk", [H, H], F32, kind="ExternalInput")
    bc = nc.dram_tensor("bc", [H, 1], F32, kind="ExternalInput")
    hm = nc.dram_tensor("hm", [H, NH], F32, kind="ExternalInput")
    identd = nc.dram_tensor("identd", [H, H], F32, kind="ExternalInput")
    oh = nc.dram_tensor("oh", [4, 2, N], F32, kind="ExternalInput")
    mnegA = nc.dram_tensor("mnegA", [4, H], F32, kind="ExternalInput")
    mnegP = nc.dram_tensor("mnegP", [4, H], F32, kind="ExternalInput")
    zer = nc.dram_tensor("zer", [H, 512], F32, kind="ExternalInput")
    seld = nc.dram_tensor("seld", [H, 32], F32, kind="ExternalInput")
    roffs = nc.dram_tensor("roffs", [B_CORE, 1], I32, kind="ExternalInput")

    probs = nc.dram_tensor("probs", [B_CORE, N], F32, kind="ExternalOutput")

    with tile.TileContext(nc) as tc:
        with (
            tc.tile_pool(name="wts", bufs=1) as wts,
            tc.tile_pool(name="xn", bufs=1) as xnp,
            tc.tile_pool(name="xt", bufs=1) as xtp,
            tc.tile_pool(name="big", bufs=4) as bigp,
            tc.tile_pool(name="et", bufs=2) as etp,
            tc.tile_pool(name="pad", bufs=2) as padp,
            tc.tile_pool(name="sm", bufs=1) as smp,
            tc.tile_pool(name="ps_big", bufs=1, space="PSUM") as psb,
            tc.tile_pool(name="ps_tr", bufs=2, space="PSUM") as pst,
            tc.tile_pool(name="ps_sm", bufs=2, space="PSUM") as pss,
        ):
            # ---------- weights & constants ----------
            def wtile(dram, shape, dtype=F32, cast_r=False, tag=None):
                t = wts.tile(shape, dtype, tag=tag or dram.name)
                if cast_r:
                    nc.sync.dma_start(r(t[:]), r(dram[:]))
                else:
                    nc.sync.dma_start(t[:], dram[:])
                return t

            wqgT_s = wtile(wqgT, [H, H])
            wsumT_s = wtile(wsumT, [H, H])
            wk_s = wtile(wk, [H, H])
            wvT_s = wtile(wvT, [H, H])
            wcT_s = wtile(wcT, [H, H])
            wlk_s = wtile(wlk, [H, H])
            bc_s = wtile(bc, [H, 1])
            hm_s = wtile(hm, [H, NH])
            id_f = wtile(identd, [H, H], tag="id_f")
            clsT_s = wtile(clsT, [H, B_CORE])
            oh_s = wtile(oh, [4, 2, N], cast_r=True)
            mnegA_s = wtile(mnegA, [4, H], cast_r=True)
            mnegP_s = wtile(mnegP, [4, H], cast_r=True)
            sel_s = wtile(seld, [H, 32])

            # ---------- X natural, quad-interleaved ----------
            # xn_q[p, j, bi, c] = x[4q + bi, j*125 + p, c]
            xn = []
            for q in range(2):
                t = xnp.tile([NP, NJ, 4 * H], F32, tag=f"xn{q}")
                for bi in range(4):
                    b = 4 * q + bi
                    nc.sync.dma_start(
                        r(t[:, :, H * bi:H * (bi + 1)]),
                        r(x[b].rearrange("(j p) c -> p j c", p=NP)),
                    )
                xn.append(t)

            # ---------- last-patch gather ----------
            roffs_s = smp.tile([B_CORE, 1], I32, tag="roffs")
            nc.sync.dma_start(roffs_s[:], roffs[:])
            le_s = smp.tile([B_CORE, H], F32, tag="le")
            nc.gpsimd.indirect_dma_start(
                out=le_s[:], out_offset=None,
                in_=x[:].rearrange("b n c -> (b n) c"),
                in_offset=bass.IndirectOffsetOnAxis(ap=roffs_s[:, :1], axis=0),
            )

            # ---------- X^T via PE transposes ----------
            xT = xtp.tile([H, B_CORE * N], F32, tag="xT")
            ncopies = 0
            for q in range(2):
                for bi in range(4):
                    b = 4 * q + bi
                    for k in range(4):
                        ps = pst.tile([H, 4, H], F32, tag="trps")
                        for i in range(4):
                            j = 4 * k + i
                            nc.tensor.transpose(
                                ps[:, i, 0:NP],
                                xn[q][0:NP, j, H * bi:H * (bi + 1)].bitcast(F32),
                                id_f[0:NP, 0:NP],
                            )
                        dst = r(xT[:, b * N + NCHUNK * k: b * N + NCHUNK * (k + 1)]
                                .rearrange("p (j n) -> p j n", n=NP))
                        src = ps[:].rearrange("p j c -> p j c")[:, :, 0:NP]
                        nc.vector.tensor_copy(dst, src)
                        ncopies += 1

            # ---------- Q path ----------
            leT_ps = pss.tile([H, B_CORE], F32, tag="smps")
            nc.tensor.transpose(leT_ps[:], le_s[:], id_f[0:B_CORE, 0:B_CORE])
            leT_s = smp.tile([H, B_CORE], F32, tag="leTs")
            nc.vector.tensor_copy(leT_s[:], leT_ps[:])

            q_ps = pss.tile([H, B_CORE], F32, tag="smps")
            nc.tensor.matmul(q_ps[:], wqgT_s[:], clsT_s[:], start=True, stop=False)
            nc.tensor.matmul(q_ps[:], wsumT_s[:], leT_s[:], start=False, stop=True)
            qT_s = smp.tile([H, B_CORE], F32, tag="qTs")
            nc.vector.tensor_copy(qT_s[:], q_ps[:])

            qtil = smp.tile([H, B_CORE * NH], F32, tag="qtil")
            for b in range(B_CORE):
                nc.vector.tensor_scalar_mul(
                    qtil[:, NH * b:NH * (b + 1)], hm_s[:], qT_s[:, b:b + 1])

            r_ps = pss.tile([H, B_CORE * NH], F32, tag="smps")
            nc.tensor.matmul(r_ps[:], wk_s[:], qtil[:], start=True, stop=True)

            # rp_q[:, bi, 32*bi + h] = R[:, (4q+bi)*8 + h], zeros elsewhere
            rp = []
            for q in range(2):
                t = padp.tile([H, 4, H], F32, tag="pad")
                nc.sync.dma_start(
                    r(t[:].rearrange("p a c -> p (a c)")), r(zer[:]))
                for bi in range(4):
                    b = 4 * q + bi
                    nc.vector.tensor_copy(
                        r(t[:, bi, 32 * bi:32 * bi + NH]),
                        r_ps[:, NH * b:NH * (b + 1)],
                    )
                rp.append(t)

            # ---------- scores -> exp -> E^T ----------
            ets = []
            rcps = []
            for q in range(2):
                sc = psb.tile([H, N], F32, tag="bigps")
                for off, ln in SCHUNKS:
                    cs = slice(off, off + ln)
                    for bi in range(4):
                        b = 4 * q + bi
                        nc.tensor.matmul(
                            sc[:, cs], r(rp[q][:, bi, :]),
                            r(xT[:, b * N + off: b * N + off + ln]),
                            start=(bi == 0), stop=False,
                        )
                    nc.tensor.matmul(
                        sc[:, cs], r(mnegA_s[:]), r(oh_s[:, q, cs]),
                        start=False, stop=True,
                    )
                e_t = bigp.tile([H, N], F32, tag="EB")
                sums = smp.tile([H, 1], F32, tag=f"sums{q}")
                nc.scalar.activation(
                    e_t[:], sc[:], mybir.ActivationFunctionType.Exp,
                    bias=0.0, scale=1.0, accum_out=sums[:],
                )
                rcp = smp.tile([H, 1], F32, tag=f"rcp{q}")
                nc.vector.reciprocal(rcp[:], sums[:])
                rcps.append(rcp)

                et = etp.tile([NP, NJ, 32], F32, tag="et")
                for k in range(4):
                    ps = pst.tile([NP, 4, H], F32, tag="trps")
                    for i in range(4):
                        j = 4 * k + i
                        nc.tensor.transpose(
                            ps[:, i, :], e_t[:, NP * j:NP * (j + 1)],
                            id_f[:],
                        )
                    # keep only cols {32*bi + h}: E rows used by this quad
                    nc.vector.tensor_copy(
                        r(et[:, 4 * k:4 * k + 4, :]
                          .rearrange("p j (a c) -> p j a c", c=NH)),
                        ps[:].rearrange("p j (a c) -> p j a c", c=32)[
                            :, :, :, 0:NH],
                    )
                ets.append(et)

            # ---------- attnV ----------
            u_s = []
            for q in range(2):
                u_ps = pss.tile([32, 512], F32, tag="smps")
                for j in range(NJ):
                    nc.tensor.matmul(
                        u_ps[:], r(ets[q][0:NP, j, :]),
                        r(xn[q][0:NP, j, :]),
                        start=(j == 0), stop=(j == NJ - 1),
                    )
                rq_ps = pss.tile([32, 1], F32, tag="smps")
                nc.tensor.matmul(
                    rq_ps[:], sel_s[:], rcps[q][:], start=True, stop=True)
                rq_s = smp.tile([32, 1], F32, tag=f"rqs{q}")
                nc.vector.tensor_copy(rq_s[:], rq_ps[:])
                ut = smp.tile([32, 512], F32, tag=f"us{q}")
                nc.vector.tensor_scalar_mul(ut[:], u_ps[:], rq_s[:])
                u_s.append(ut)

            Ut = smp.tile([H, NH, B_CORE], F32, tag="Ut")
            for q in range(2):
                for bi in range(4):
                    b = 4 * q + bi
                    utp = pss.tile([H, 32], F32, tag="smps")
                    nc.tensor.transpose(
                        utp[:], u_s[q][0:32, H * bi:H * (bi + 1)],
                        id_f[0:32, 0:32],
                    )
                    nc.vector.tensor_copy(
                        Ut[:, :, b], utp[:, NH * bi:NH * (bi + 1)])

            v_ps = pss.tile([B_CORE, H], F32, tag="smps")
            for h in range(NH):
                nc.tensor.matmul(
                    v_ps[:, HD * h:HD * (h + 1)], Ut[:, h, :],
                    wvT_s[:, HD * h:HD * (h + 1)],
                    start=True, stop=True,
                )
            v_s = smp.tile([B_CORE, H], F32, tag="vs")
            nc.vector.tensor_copy(v_s[:], v_ps[:])

            vt_ps = pss.tile([H, B_CORE], F32, tag="smps")
            nc.tensor.transpose(vt_ps[:], v_s[:], id_f[0:B_CORE, 0:B_CORE])
            vt_s = smp.tile([H, B_CORE], F32, tag="vts")
            nc.vector.tensor_copy(vt_s[:], vt_ps[:])
            fq_ps = pss.tile([H, B_CORE], F32, tag="smps")
            nc.tensor.matmul(fq_ps[:], wcT_s[:], vt_s[:], start=True, stop=True)
            fq_s = smp.tile([H, B_CORE], F32, tag="fqs")
            nc.scalar.activation(
                fq_s[:], fq_ps[:], mybir.ActivationFunctionType.Identity,
                bias=bc_s[:, 0:1], scale=1.0,
            )

            w2_ps = pss.tile([H, B_CORE], F32, tag="smps")
            nc.tensor.matmul(w2_ps[:], wlk_s[:], fq_s[:], start=True, stop=True)
            wp = []
            for q in range(2):
                t = padp.tile([H, 4, H], F32, tag="pad")
                nc.sync.dma_start(
                    r(t[:].rearrange("p a c -> p (a c)")), r(zer[:]))
                for bi in range(4):
                    b = 4 * q + bi
                    nc.vector.tensor_copy(
                        r(t[:, bi, 32 * bi:32 * bi + 1]), w2_ps[:, b:b + 1])
                wp.append(t)

            # ---------- pointer scores + final softmax ----------
            for q in range(2):
                ps2 = psb.tile([H, N], F32, tag="bigps")
                for off, ln in SCHUNKS:
                    cs = slice(off, off + ln)
                    for bi in range(4):
                        b = 4 * q + bi
                        nc.tensor.matmul(
                            ps2[:, cs], r(wp[q][:, bi, :]),
                            r(xT[:, b * N + off: b * N + off + ln]),
                            start=(bi == 0), stop=False,
                        )
                    nc.tensor.matmul(
                        ps2[:, cs], r(mnegP_s[:]), r(oh_s[:, q, cs]),
                        start=False, stop=True,
                    )
                t_t = bigp.tile([H, N], F32, tag="EB")
                nc.scalar.activation(
                    t_t[:], ps2[:], mybir.ActivationFunctionType.Tanh)
                e2_t = bigp.tile([H, N], F32, tag="EB")
                s2 = smp.tile([H, 1], F32, tag=f"s2{q}")
                nc.scalar.activation(
                    e2_t[:], t_t[:], mybir.ActivationFunctionType.Exp,
                    bias=0.0, scale=10.0, accum_out=s2[:],
                )
                rcp2 = smp.tile([H, 1], F32, tag=f"rcp2{q}")
                nc.vector.reciprocal(rcp2[:], s2[:])
                nc.vector.tensor_scalar_mul(e2_t[:], e2_t[:], rcp2[:])
                nc.sync.dma_start(
                    probs[4 * q:4 * q + 4, :],
                    e2_t[:].rearrange("(a b) f -> a b f", b=32)[:, 0, :],
                )

    nc.compile()
    return nc


def _prep_inputs(patch_embeddings, fixed_content_cls, Wq_graph, Wq_first,
                 Wq_last, Wk, Wv, logit_Wk, Wc, bc, last_patch):
    qs = 1.0 / math.sqrt(HD)
    ls = 1.0 / math.sqrt(H)
    f32 = lambda a: np.ascontiguousarray(a, dtype=np.float32)
    shared = {
        "wqgT": f32(np.asarray(Wq_graph).T * qs),
        "wsumT": f32((np.asarray(Wq_first) + np.asarray(Wq_last)).T * qs),
        "wk": f32(Wk),
        "wvT": f32(np.asarray(Wv).T),
        "wcT": f32(np.asarray(Wc).T),
        "wlk": f32(np.asarray(logit_Wk) * ls),
        "bc": f32(np.asarray(bc)[:, None]),
        "identd": np.eye(H, dtype=np.float32),
        "seld": np.eye(H, dtype=np.float32)
            .reshape(H, 4, 32)[:, :, :8].reshape(H, 32),
        "zer": np.zeros((H, 512), np.float32),
    }
    hm = np.zeros((H, NH), np.float32)
    for h in range(NH):
        hm[HD * h:HD * (h + 1), h] = 1.0
    shared["hm"] = hm

    mnegA = np.zeros((4, H), np.float32)
    mnegP = np.zeros((4, H), np.float32)
    for bi in range(4):
        mnegA[bi, 32 * bi:32 * bi + NH] = -1e9
        mnegP[bi, 32 * bi] = -1e9
    shared["mnegA"] = mnegA
    shared["mnegP"] = mnegP

    pe = np.asarray(patch_embeddings)
    cls = np.asarray(fixed_content_cls)
    lp = np.asarray(last_patch).astype(np.int64)
    in_maps = []
    for c in range(N_CORES):
        bs = slice(B_CORE * c, B_CORE * (c + 1))
        lp_c = lp[bs]
        ohc = np.zeros((4, 2, N), np.float32)
        for b in range(B_CORE):
            ohc[b % 4, b // 4, lp_c[b]] = 1.0
        m = dict(shared)
        m["x"] = f32(pe[bs])
        m["clsT"] = f32(cls[bs, 0, :].T)
        m["oh"] = ohc
        m["roffs"] = (np.arange(B_CORE) * N + lp_c).astype(np.int32)[:, None]
        in_maps.append(m)
    return in_maps


def kernel(trace=False, **inputs):
    if "nc" not in _CACHE:
        _CACHE["nc"] = build()
    nc = _CACHE["nc"]
    in_maps = _prep_inputs(**inputs)
    res = run_bass_kernel_spmd(nc, in_maps, list(range(N_CORES)), trace=trace)
    out = np.concatenate([res.results[c]["probs"] for c in range(N_CORES)], axis=0)
    if trace:
        return out, res
    return out

